# revision 1
# baseline (speedup 1.0000x reference)
"""Distributed Trainium2 Bass kernel for a single attention head.

Reference computation (fp32 jax):
    q = queries @ Wq.T + bq        # [B,S,Df]
    k = keys    @ Wk.T + bk
    v = values  @ Wv.T + bv
    attn = softmax((q @ k.T) / sqrt(Df), axis=-1)
    out  = attn @ v                # [B,S,Df]

with B=4, S=4096, D_MODEL=1024, D_FEATURE=64.

Sharding: 8 cores = (batch b in 0..3) x (query-half h in 0..1).
Core c handles batch b=c//2, q rows [h*2048, (h+1)*2048). Each core gets
its q-half plus the FULL keys/values of its batch (no collectives), all
pre-transposed on the host to m-contraction-major layout and converted
to bf16 so matmuls run at full PE rate and DMA bytes are halved.

Kernel structure (per core):
  - inputs arrive i-block-major: [128, nblk * (8 m-chunks * 512 cols)]
    so each 512-column projection block is one contiguous 1MB DMA and
    projection of block i can start as soon as its DMA lands.
  - projections: psum[64, 512] accumulated over 8 m-chunks,
    lhsT = wT chunk [128, 64], rhs = xT chunk [128, 512]; DVE evicts
    psum -> bf16 SBUF with the per-feature bias added.
  - scores are computed TRANSPOSED, flash-style: ST[j, i]
    (lhsT = kT[64, jc*128:...], rhs = qT[64, i-chunk]) so softmax-exp
    input and the attn@v moving operand are both natural layout.
  - attention runs in TWO i-passes of 1024 q-rows each so that the
    scores psum can double-buffer (2x[128,1024] = 4 banks) next to the
    out.T accumulator ([65,1024] = 2 banks) within the 8 PSUM banks.
    Pass A is interleaved with the k/v projections at k-block
    granularity so the ACT engine starts exp-ing ~8us into the kernel.
  - exp is fused with the 1/8 score scale on ACT; output PT is bf16.
  - attn@v accumulates out.T[f, i] with lhsT = v_aug[j, 65] (v in
    natural [j, f] layout + ones column -> row 64 of out.T is the
    softmax denominator for free).
  - finals per pass: evict out.T, PE-transpose 128-row chunks,
    reciprocal of the denominator column, scale, DMA out fp32 rows.
"""

import numpy as np
import ml_dtypes

import concourse.bass as bass
import concourse.mybir as mybir
import concourse.tile as tile
from concourse import bacc
from concourse.bass_utils import run_bass_kernel_spmd
from concourse.masks import make_identity

B = 4
S = 4096
DM = 1024
DF = 64
NCORES = 8
SQ = S // 2          # local q rows per core
MC = DM // 128       # 8 contraction chunks
NI = 512             # moving-operand tile (one PSUM bank of fp32)
JC = S // 128        # 32 key chunks
NBQ = SQ // NI       # 4 q column blocks
NBK = S // NI        # 8 k/v column blocks
IP = SQ // 2         # 1024: i-rows per attention pass
WB = 5 * DF          # per-m-chunk weight columns: [wq|wq|wk|wk|wv]
BF16 = mybir.dt.bfloat16
F32 = mybir.dt.float32
NP_BF16 = ml_dtypes.bfloat16
EXP = mybir.ActivationFunctionType.Exp


def build_kernel(tc):
    nc = tc.nc
    xq = nc.dram_tensor("xq", [128, NBQ * MC * NI], BF16, kind="ExternalInput")
    xk = nc.dram_tensor("xk", [128, NBK * MC * NI], BF16, kind="ExternalInput")
    xv = nc.dram_tensor("xv", [128, NBK * MC * NI], BF16, kind="ExternalInput")
    wT = nc.dram_tensor("wT", [128, MC * WB], BF16, kind="ExternalInput")
    bias = nc.dram_tensor("bias", [128, 3], F32, kind="ExternalInput")
    out = nc.dram_tensor("out", [SQ, DF], F32, kind="ExternalOutput")

    from contextlib import ExitStack

    with ExitStack() as ctx:
        const_pool = ctx.enter_context(tc.tile_pool(name="const", bufs=1))
        xin_pool = ctx.enter_context(tc.tile_pool(name="xin", bufs=9))
        act_pool = ctx.enter_context(tc.tile_pool(name="act", bufs=1))
        pt_pool = ctx.enter_context(tc.tile_pool(name="pt", bufs=4))
        outT_pool = ctx.enter_context(tc.tile_pool(name="outT", bufs=1))
        fin_pool = ctx.enter_context(tc.tile_pool(name="fin", bufs=2))
        # PSUM budget (8 banks): ppsum 2x[64,512] = 2 banks (proj psum,
        # also vtrans/finals scratch), spsum 2x[128,1024] = 4 banks
        # (scores double-buffer), opsum 2 banks (warmup scratch, then the
        # per-pass [65,1024] out.T accumulator).
        ppsum = ctx.enter_context(tc.tile_pool(name="ppsum", bufs=2, space="PSUM"))
        spsum = ctx.enter_context(tc.tile_pool(name="spsum", bufs=2, space="PSUM"))
        opsum = ctx.enter_context(tc.tile_pool(name="opsum", bufs=1, space="PSUM"))

        # ---- constants (DMA'd first) ----
        wT_sb = const_pool.tile([128, MC * WB], BF16, tag="wt")
        nc.sync.dma_start(wT_sb[:], wT[:])
        bias_sb = const_pool.tile([128, 3], F32, tag="bias")
        nc.sync.dma_start(bias_sb[:], bias[:])
        # preload the ACT exp table while DMAs stream
        scratch = const_pool.tile([DF, 1], F32, tag="scratch")
        nc.scalar.activation(scratch[:], bias_sb[0:DF, 0:1], EXP)
        ident = const_pool.tile([128, 128], BF16, tag="ident")
        make_identity(nc, ident[:])
        identf = const_pool.tile([128, 128], F32, tag="identf")
        make_identity(nc, identf[:])

        # ---- PE warm-up: ~7us of dummy matmuls so the HAM clock gate
        # opens (1.2 -> 2.4 GHz) and stays open until the first real
        # matmul's input DMA lands ----
        warm = opsum.tile([DF, 128], F32, tag="po")
        for _ in range(96):
            nc.tensor.matmul(warm[:], ident[:, 0:DF], ident[:], start=True, stop=True)

        # ---- input DMAs, interleaved q first then k/v alternating ----
        def load_block(x_dram, i):
            t = xin_pool.tile([128, MC * NI], BF16, tag="xin")
            nc.sync.dma_start(t[:], x_dram[:, i * MC * NI:(i + 1) * MC * NI])
            return t

        q_tiles = [load_block(xq, i) for i in range(NBQ)]
        kv_tiles = {}
        for i in range(NBK):
            kv_tiles[("k", i)] = load_block(xk, i)
            kv_tiles[("v", i)] = load_block(xv, i)

        # ---- persistent activations ----
        # q/k projections land duplicated in both partition halves so the
        # score matmuls can run pair-wise on independent 64-row PE tiles
        qT_sb = act_pool.tile([128, SQ], BF16, tag="qT")
        kT_sb = act_pool.tile([128, S], BF16, tag="kT")
        vT_sb = act_pool.tile([DF, S], BF16, tag="vT")
        v_sb = act_pool.tile([128, JC * (DF + 1)], BF16, tag="v")  # [128, 32*65]
        nc.gpsimd.memset(v_sb[:], 1.0)  # col DF of every block stays 1.0

        def w_slice(mc_i, which):
            # which: 0 = [wq|wq], 1 = [wk|wk] (128-wide dup), 2 = wv (64)
            o = mc_i * WB + which * 2 * DF
            return wT_sb[:, o:o + (2 * DF if which < 2 else DF)]

        def project_block(x_tile, i, which, dest_sb, bias_col):
            """One 512-column projection block accumulated over 8 m-chunks."""
            rows = 2 * DF if which < 2 else DF
            ps = ppsum.tile([rows, NI], F32, tag="ps")
            for mc_i in range(MC):
                nc.tensor.matmul(
                    ps[:], w_slice(mc_i, which), x_tile[:, mc_i * NI:(mc_i + 1) * NI],
                    start=(mc_i == 0), stop=(mc_i == MC - 1),
                )
            nc.vector.tensor_scalar_add(
                dest_sb[:, i * NI:(i + 1) * NI], ps[:],
                bias_sb[0:rows, bias_col:bias_col + 1])

        # ---- q projection up front ----
        for i in range(NBQ):
            project_block(q_tiles[i], i, 0, qT_sb, 0)

        # pass-B exp results are computed during pass A and parked in SBUF
        ptb_sb = act_pool.tile([128, JC * IP], BF16, tag="ptb")  # 8 MB

        def attn_pair(jc0, poA):
            """Scores + exp for BOTH i-halves of TWO key chunks; the two
            chunks' score matmuls run on independent 64-row PE tiles
            (partitions 0-63 / 64-127 of the duplicated qT/kT), so they
            stream concurrently. attn@v for i-half A follows immediately;
            i-half B's exp output parks in ptb_sb."""
            for ipass in range(2):
                io = ipass * IP
                ss0 = spsum.tile([128, IP], F32, tag="ss", name="ss0")
                ss1 = spsum.tile([128, IP], F32, tag="ss", name="ss1")
                sss = [ss0, ss1]
                for ii in range(IP // NI):
                    for t in range(2):
                        jc = jc0 + t
                        p0 = t * DF
                        nc.tensor.matmul(
                            sss[t][:, ii * NI:(ii + 1) * NI],
                            kT_sb[p0:p0 + DF, jc * 128:(jc + 1) * 128],
                            qT_sb[p0:p0 + DF, io + ii * NI:io + (ii + 1) * NI],
                            start=True, stop=True,
                        )
                for t in range(2):
                    jc = jc0 + t
                    if ipass == 0:
                        pts = pt_pool.tile([128, IP], BF16, tag="pt")
                    else:
                        pts = ptb_sb[:, jc * IP:(jc + 1) * IP]
                    nc.scalar.activation(pts[:], sss[t][:], EXP, scale=0.125)
                    if ipass == 0:
                        for ii in range(IP // NI):
                            nc.tensor.matmul(
                                poA[:, ii * NI:(ii + 1) * NI],
                                v_sb[:, jc * (DF + 1):(jc + 1) * (DF + 1)],
                                pts[:, ii * NI:(ii + 1) * NI],
                                start=(jc == 0), stop=(jc == JC - 1),
                            )

        def finals_chunk(ipass, outT_sb, ob, c):
            pf = ppsum.tile([128, DF + 1], F32, tag="ps")
            nc.tensor.transpose(
                pf[:], outT_sb[:, c * 128:(c + 1) * 128],
                identf[0:DF + 1, 0:DF + 1])
            rcp = fin_pool.tile([128, 1], F32, tag="rcp")
            nc.vector.reciprocal(rcp[:], pf[:, DF:DF + 1])
            nc.vector.tensor_scalar_mul(ob[:, c, :], pf[:, 0:DF], rcp[:])

        def finals_store(ipass, ob):
            # one strided DMA for all 1024 rows of this i-half
            nc.sync.dma_start(
                out[ipass * IP:(ipass + 1) * IP, :].rearrange(
                    "(c p) f -> p c f", p=128),
                ob[:])

        # ---- pass A: k/v projection interleaved with scores/exp for both
        # i-halves + attn@v for i-half A ----
        poA = opsum.tile([DF + 1, IP], F32, tag="po")
        for kb in range(NBK):
            project_block(kv_tiles[("k", kb)], kb, 1, kT_sb, 1)
            project_block(kv_tiles[("v", kb)], kb, 2, vT_sb, 2)
            for jc in range(4 * kb, 4 * kb + 4):
                pv = ppsum.tile([128, DF], BF16, tag="ps")
                nc.tensor.transpose(
                    pv[:], vT_sb[:, jc * 128:(jc + 1) * 128], ident[0:DF, 0:DF])
                nc.vector.tensor_copy(
                    v_sb[:, jc * (DF + 1):jc * (DF + 1) + DF], pv[:])
            for jc0 in range(4 * kb, 4 * kb + 4, 2):
                attn_pair(jc0, poA)

        # ---- pass B: attn@v for i-half B from parked exp outputs; pass A
        # finals are interleaved to fill PE gaps. ----
        outT_A = outT_pool.tile([DF + 1, IP], F32, tag="outT")
        obA = fin_pool.tile([128, IP // 128, DF], F32, tag="ob")
        nc.vector.tensor_copy(outT_A[:], poA[:])
        poB = opsum.tile([DF + 1, IP], F32, tag="po")
        for jc in range(JC):
            for ii in range(IP // NI):
                nc.tensor.matmul(
                    poB[:, ii * NI:(ii + 1) * NI],
                    v_sb[:, jc * (DF + 1):(jc + 1) * (DF + 1)],
                    ptb_sb[:, jc * IP + ii * NI:jc * IP + (ii + 1) * NI],
                    start=(jc == 0), stop=(jc == JC - 1),
                )
            if jc % 4 == 3:
                finals_chunk(0, outT_A, obA, jc // 4)
        finals_store(0, obA)

        outT_B = outT_pool.tile([DF + 1, IP], F32, tag="outT")
        obB = fin_pool.tile([128, IP // 128, DF], F32, tag="ob")
        nc.vector.tensor_copy(outT_B[:], poB[:])
        for c in range(IP // 128):
            finals_chunk(1, outT_B, obB, c)
        finals_store(1, obB)


_COMPILED = None


def get_compiled():
    global _COMPILED
    if _COMPILED is None:
        nc = bacc.Bacc("TRN2", target_bir_lowering=False, debug=False,
                       enable_asserts=False, num_devices=NCORES)
        with tile.TileContext(nc) as tc:
            build_kernel(tc)
        nc.compile()
        _COMPILED = nc
    return _COMPILED


def _to_block_major(xT):
    """[DM, s_len] -> [128, nblk*MC*NI]: 512-col blocks, m-chunk-major inside."""
    s_len = xT.shape[1]
    nblk = s_len // NI
    # (mc, p, blk, s) -> (p, blk, mc, s)
    return np.ascontiguousarray(
        xT.reshape(MC, 128, nblk, NI).transpose(1, 2, 0, 3).reshape(128, nblk * MC * NI))


def make_in_maps(queries, keys, values, Wq, bq, Wk, bk, Wv, bv):
    queries = np.asarray(queries, dtype=np.float32)
    keys = np.asarray(keys, dtype=np.float32)
    values = np.asarray(values, dtype=np.float32)
    WqT, WkT, WvT = np.asarray(Wq).T, np.asarray(Wk).T, np.asarray(Wv).T
    wT_full = np.concatenate([WqT, WqT, WkT, WkT, WvT], axis=1)  # [DM, 320]
    wT_host = np.ascontiguousarray(
        wT_full.reshape(MC, 128, WB).transpose(1, 0, 2).reshape(128, MC * WB)
    ).astype(NP_BF16)
    bias64 = np.stack(
        [np.asarray(bq), np.asarray(bk), np.asarray(bv)], axis=1
    ).astype(np.float32)
    bias_host = np.concatenate([bias64, bias64], axis=0)  # [128, 3]

    in_maps = []
    for c in range(NCORES):
        b, h = c // 2, c % 2
        in_maps.append({
            "xq": _to_block_major(queries[b, h * SQ:(h + 1) * SQ, :].T).astype(NP_BF16),
            "xk": _to_block_major(keys[b].T).astype(NP_BF16),
            "xv": _to_block_major(values[b].T).astype(NP_BF16),
            "wT": wT_host, "bias": bias_host,
        })
    return in_maps


def assemble(results):
    out = np.zeros((B, S, DF), dtype=np.float32)
    for c in range(NCORES):
        b, h = c // 2, c % 2
        out[b, h * SQ:(h + 1) * SQ, :] = results[c]["out"]
    return out


def kernel(**inputs):
    nc = get_compiled()
    in_maps = make_in_maps(**inputs)
    res = run_bass_kernel_spmd(nc, in_maps, core_ids=list(range(NCORES)))
    return assemble(res.results)



# revision 3
# speedup vs baseline: 1.0809x; 1.0809x over previous
"""Distributed Trainium2 Bass kernel for a single attention head.

Reference computation (fp32 jax):
    q = queries @ Wq.T + bq        # [B,S,Df]
    k = keys    @ Wk.T + bk
    v = values  @ Wv.T + bv
    attn = softmax((q @ k.T) / sqrt(Df), axis=-1)
    out  = attn @ v                # [B,S,Df]

with B=4, S=4096, D_MODEL=1024, D_FEATURE=64.

Sharding: 8 cores = (batch b in 0..3) x (query-half h in 0..1).
Core c handles batch b=c//2, q rows [h*2048, (h+1)*2048). Each core gets
its q-half plus the FULL keys/values of its batch (no collectives).

v2 design notes (changes vs the bf16 baseline):
  - q/k projections run as fp8e4 DoubleRow matmuls (2 contraction
    chunks per PE pass): x_q/x_k stream from HBM in fp8 (half the DMA
    bytes) pre-interleaved on the host as [128, pair, slot, 512];
    weights likewise [128, pair, slot, 128] with the [w|w] partition
    duplication kept so scores can pair on independent 64-row PE
    tiles. v projection (and its inputs) stays bf16: an fp8 v path
    pushes the output error over the 2e-2 budget (v-quantization noise
    lands on the output un-averaged).
  - ACT is the critical engine (64 exp instructions of [128,1024] =
    ~70us busy). DMA order (wq,wk,xq*,xk0 first) + fp8 q/k proj get
    the first exp issued ~6us in, and the kb loop paces everything
    else to keep ACT gap-free.
  - attn@v runs in 4 accumulation generations so almost all of it
    overlaps the exp stream instead of trailing it:
      G1 = i-half A x keys 0..15   (opsum, during kb0-3)
      G2 = i-half B x keys 0..15   (opsum, during kb4-5, parked pts)
      G3 = i-half A x keys 16..31  (opsum, during kb6-7)
      G4 = i-half B x keys 16..31  (2x[65,512] in the ppsum banks the
           projections no longer need after kb7)
    exp outputs are parked in SBUF for every B-half chunk and for the
    A-half of chunks 16..23 (consumed one generation later).
  - out.T accumulators merge via DVE adds; finals (PE transpose,
    reciprocal of the ones-column denominator, scale, strided DMA)
    run once per i-half at the end.
"""

import numpy as np
import ml_dtypes

import concourse.bass as bass
import concourse.mybir as mybir
import concourse.tile as tile
from concourse import bacc
from concourse.bass_utils import run_bass_kernel_spmd
from concourse.masks import make_identity

B = 4
S = 4096
DM = 1024
DF = 64
NCORES = 8
SQ = S // 2          # local q rows per core
MC = DM // 128       # 8 contraction chunks
MCP = MC // 2        # 4 DoubleRow chunk-pairs
NI = 512             # moving-operand tile (one PSUM bank of fp32)
JC = S // 128        # 32 key chunks
NBQ = SQ // NI       # 4 q column blocks
NBK = S // NI        # 8 k/v column blocks
IP = SQ // 2         # 1024: i-rows per attention half
BF16 = mybir.dt.bfloat16
F8 = mybir.dt.float8e4
F32 = mybir.dt.float32
NP_BF16 = ml_dtypes.bfloat16
NP_F8 = ml_dtypes.float8_e4m3
EXP = mybir.ActivationFunctionType.Exp
DR = mybir.MatmulPerfMode.DoubleRow

# parked exp outputs: all 32 B-half chunks + A-half chunks 16..23
NPARK = JC + 8


def _park_idx(jc, ipass):
    if ipass == 1:
        return jc
    assert 16 <= jc < 24
    return JC + (jc - 16)


def build_kernel(tc):
    nc = tc.nc
    xq = nc.dram_tensor("xq", [128, NBQ * MCP * 2 * NI], F8, kind="ExternalInput")
    xk = nc.dram_tensor("xk", [128, NBK * MCP * 2 * NI], F8, kind="ExternalInput")
    xv = nc.dram_tensor("xv", [128, NBK * MC * NI], BF16, kind="ExternalInput")
    wq = nc.dram_tensor("wq", [128, MCP * 2 * 128], F8, kind="ExternalInput")
    wk = nc.dram_tensor("wk", [128, MCP * 2 * 128], F8, kind="ExternalInput")
    wv = nc.dram_tensor("wv", [128, MC * DF], BF16, kind="ExternalInput")
    bias = nc.dram_tensor("bias", [128, 3], F32, kind="ExternalInput")
    out = nc.dram_tensor("out", [SQ, DF], F32, kind="ExternalOutput")

    from contextlib import ExitStack

    with ExitStack() as ctx:
        const_pool = ctx.enter_context(tc.tile_pool(name="const", bufs=1))
        xq_pool = ctx.enter_context(tc.tile_pool(name="xq", bufs=4))
        xk_pool = ctx.enter_context(tc.tile_pool(name="xk", bufs=3))
        xv_pool = ctx.enter_context(tc.tile_pool(name="xv", bufs=3))
        act_pool = ctx.enter_context(tc.tile_pool(name="act", bufs=1))
        pt_pool = ctx.enter_context(tc.tile_pool(name="pt", bufs=4))
        outT_pool = ctx.enter_context(tc.tile_pool(name="outT", bufs=2))
        fin_pool = ctx.enter_context(tc.tile_pool(name="fin", bufs=2))
        # PSUM budget (8 banks): ppsum 2x[<=128,512] = 2 banks (proj
        # psum + vtrans scratch, then the G4 accumulator pair + finals
        # scratch), spsum 2x[128,1024] = 4 banks (scores double-buffer),
        # opsum [65,1024] = 2 banks (G1/G2/G3 in sequence).
        ppsum = ctx.enter_context(tc.tile_pool(name="ppsum", bufs=2, space="PSUM"))
        spsum = ctx.enter_context(tc.tile_pool(name="spsum", bufs=2, space="PSUM"))
        opsum = ctx.enter_context(tc.tile_pool(name="opsum", bufs=1, space="PSUM"))

        # ---- constants (DMA'd first) ----
        wq_sb = const_pool.tile([128, MCP * 2 * 128], F8, tag="wq")
        nc.sync.dma_start(wq_sb[:], wq[:])
        wk_sb = const_pool.tile([128, MCP * 2 * 128], F8, tag="wk")
        nc.sync.dma_start(wk_sb[:], wk[:])
        bias_sb = const_pool.tile([128, 3], F32, tag="bias")
        nc.sync.dma_start(bias_sb[:], bias[:])
        wv_sb = const_pool.tile([128, MC * DF], BF16, tag="wv")
        nc.sync.dma_start(wv_sb[:], wv[:])
        # preload the ACT exp table while DMAs stream
        scratch = const_pool.tile([DF, 1], F32, tag="scratch")
        nc.scalar.activation(scratch[:], bias_sb[0:DF, 0:1], EXP)
        ident = const_pool.tile([128, 128], BF16, tag="ident")
        make_identity(nc, ident[:])
        identf = const_pool.tile([128, 128], F32, tag="identf")
        make_identity(nc, identf[:])

        # ---- PE warm-up: dummy matmuls cover the HAM ramp until the
        # first q-projection's input DMA lands ----
        warm = opsum.tile([DF, 128], F32, tag="po")
        for _ in range(24):
            nc.tensor.matmul(warm[:], ident[:, 0:DF], ident[:], start=True, stop=True)

        # ---- input DMAs: q (and k block 0) first so scores/exp start
        # as early as possible ----
        def load_block(pool, x_dram, i, width, dtype):
            t = pool.tile([128, width], dtype, tag="xin")
            nc.sync.dma_start(t[:], x_dram[:, i * width:(i + 1) * width])
            return t

        QW = MCP * 2 * NI   # 4096 fp8 elements per q/k block
        q_tiles = [load_block(xq_pool, xq, i, QW, F8) for i in range(NBQ)]
        k_tiles = {}
        v_tiles = {}
        k_tiles[0] = load_block(xk_pool, xk, 0, QW, F8)
        for i in range(NBK):
            if i > 0:
                k_tiles[i] = load_block(xk_pool, xk, i, QW, F8)
            v_tiles[i] = load_block(xv_pool, xv, i, MC * NI, BF16)

        # ---- persistent activations ----
        qT_sb = act_pool.tile([128, SQ], BF16, tag="qT")
        kT_sb = act_pool.tile([128, S], BF16, tag="kT")
        vT_sb = act_pool.tile([DF, S], BF16, tag="vT")
        v_sb = act_pool.tile([128, JC * (DF + 1)], BF16, tag="v")  # [128, 32*65]
        nc.gpsimd.memset(v_sb[:], 1.0)  # col DF of every block stays 1.0
        park_sb = act_pool.tile([128, NPARK * IP], BF16, tag="park")  # 10 MB

        def project_block_dr(x_tile, i, w_sb, dest_sb, bias_col):
            """One 512-col q/k projection block: 4 fp8 DoubleRow matmuls."""
            ps = ppsum.tile([128, NI], F32, tag="ps")
            for p in range(MCP):
                nc.tensor.matmul(
                    ps[:],
                    w_sb[:, p * 256:(p + 1) * 256].rearrange("p (s c) -> p s c", s=2),
                    x_tile[:, p * 2 * NI:(p + 1) * 2 * NI].rearrange(
                        "p (s c) -> p s c", s=2),
                    start=(p == 0), stop=(p == MCP - 1),
                    perf_mode=DR,
                )
            nc.vector.tensor_scalar_add(
                dest_sb[:, i * NI:(i + 1) * NI], ps[:],
                bias_sb[0:128, bias_col:bias_col + 1])

        def project_block_v(x_tile, i):
            """One 512-col v projection block: 8 bf16 matmuls."""
            ps = ppsum.tile([DF, NI], F32, tag="ps")
            for mc_i in range(MC):
                nc.tensor.matmul(
                    ps[:], wv_sb[:, mc_i * DF:(mc_i + 1) * DF],
                    x_tile[:, mc_i * NI:(mc_i + 1) * NI],
                    start=(mc_i == 0), stop=(mc_i == MC - 1),
                )
            nc.vector.tensor_scalar_add(
                vT_sb[:, i * NI:(i + 1) * NI], ps[:], bias_sb[0:DF, 2:3])

        # ---- q projection up front ----
        for i in range(NBQ):
            project_block_dr(q_tiles[i], i, wq_sb, qT_sb, 0)

        def v_slice(jc):
            return v_sb[:, jc * (DF + 1):jc * (DF + 1) + DF + 1]

        def attnv_mms(po, jc, src, first, last, col_off=0, ncols=IP):
            """attn@v MMs for one key chunk into accumulator po."""
            for ii in range(ncols // NI):
                nc.tensor.matmul(
                    po[:, col_off + ii * NI:col_off + (ii + 1) * NI],
                    v_slice(jc),
                    src[:, ii * NI:(ii + 1) * NI],
                    start=first, stop=last,
                )

        def attn_pair(jc0, hooks):
            """Scores + exp for BOTH i-halves of key chunks jc0, jc0+1.
            hooks(jc, ipass, pts_ap) is called after each exp to emit
            immediately-consuming attn@v matmuls."""
            for ipass in range(2):
                io = ipass * IP
                ss0 = spsum.tile([128, IP], F32, tag="ss", name="ss0")
                ss1 = spsum.tile([128, IP], F32, tag="ss", name="ss1")
                sss = [ss0, ss1]
                for ii in range(IP // NI):
                    for t in range(2):
                        jc = jc0 + t
                        p0 = t * DF
                        nc.tensor.matmul(
                            sss[t][:, ii * NI:(ii + 1) * NI],
                            kT_sb[p0:p0 + DF, jc * 128:(jc + 1) * 128],
                            qT_sb[p0:p0 + DF, io + ii * NI:io + (ii + 1) * NI],
                            start=True, stop=True,
                        )
                for t in range(2):
                    jc = jc0 + t
                    parked = (ipass == 1) or (16 <= jc < 24)
                    if parked:
                        idx = _park_idx(jc, ipass)
                        pts = park_sb[:, idx * IP:(idx + 1) * IP]
                    else:
                        pts = pt_pool.tile([128, IP], BF16, tag="pt")
                    nc.scalar.activation(pts[:], sss[t][:], EXP, scale=0.125)
                    hooks(jc, ipass, pts)

        # ================= kb loop =================
        # G1: i-half A, chunks 0..15 -> opsum (kb0-3)
        # G2: i-half B, chunks 0..15 -> opsum (kb4-5, parked pts)
        # G3: i-half A, chunks 16..31 -> opsum (kb6-7; 16..23 parked)
        # G4: i-half B, chunks 16..31 -> 2x[65,512] in ppsum (kb7+)
        poG = {}
        outT_A = outT_pool.tile([DF + 1, IP], F32, tag="outT", name="outT_A")
        outT_B = outT_pool.tile([DF + 1, IP], F32, tag="outT", name="outT_B")

        def emit_g2(jcs):
            for jc in jcs:
                attnv_mms(poG["G2"], jc, park_sb[:, jc * IP:(jc + 1) * IP],
                          first=(jc == 0), last=(jc == 15))

        def emit_g4(jcs):
            for jc in jcs:
                src = park_sb[:, jc * IP:(jc + 1) * IP]
                for ii in range(2):
                    nc.tensor.matmul(
                        poG["G4"][ii][:],
                        v_slice(jc),
                        src[:, ii * NI:(ii + 1) * NI],
                        start=(jc == 16), stop=(jc == 31),
                    )

        def hooks(jc, ipass, pts):
            if ipass == 1:
                if jc >= 28:
                    emit_g4([jc])
                return
            if jc < 16:
                attnv_mms(poG["G1"], jc, pts, first=(jc == 0), last=(jc == 15))
            elif jc >= 24:
                attnv_mms(poG["G3"], jc, pts, first=(jc == 16), last=(jc == 31))

        for kb in range(NBK):
            project_block_dr(k_tiles[kb], kb, wk_sb, kT_sb, 1)
            project_block_v(v_tiles[kb], kb)
            for jc in range(4 * kb, 4 * kb + 4):
                pv = ppsum.tile([128, DF], BF16, tag="ps")
                nc.tensor.transpose(
                    pv[:], vT_sb[:, jc * 128:(jc + 1) * 128], ident[0:DF, 0:DF])
                nc.vector.tensor_copy(
                    v_sb[:, jc * (DF + 1):jc * (DF + 1) + DF], pv[:])

            if kb == 0:
                poG["G1"] = opsum.tile([DF + 1, IP], F32, tag="po", name="poG1")
            if kb == 4:
                # G1 complete: evict and hand the opsum slot to G2
                nc.vector.tensor_copy(outT_A[:], poG["G1"][:])
                poG["G2"] = opsum.tile([DF + 1, IP], F32, tag="po", name="poG2")
            if kb == 6:
                # G2 complete: evict, slot to G3; replay parked A 16..23
                nc.vector.tensor_copy(outT_B[:], poG["G2"][:])
                poG["G3"] = opsum.tile([DF + 1, IP], F32, tag="po", name="poG3")
                for jc in range(16, 24):
                    idx = _park_idx(jc, 0)
                    attnv_mms(poG["G3"], jc,
                              park_sb[:, idx * IP:(idx + 1) * IP],
                              first=(jc == 16), last=(jc == 31))
            if kb == 7:
                # projections done: ppsum banks become the G4 accumulators
                g4a = ppsum.tile([DF + 1, NI], F32, tag="ps", name="g4a")
                g4b = ppsum.tile([DF + 1, NI], F32, tag="ps", name="g4b")
                poG["G4"] = [g4a, g4b]
                emit_g4(range(16, 28))

            for jc0 in range(4 * kb, 4 * kb + 4, 2):
                attn_pair(jc0, hooks)
                if kb in (4, 5):
                    emit_g2(range(8 * (kb - 4) + 4 * (jc0 % 4 == 2),
                                  8 * (kb - 4) + 4 + 4 * (jc0 % 4 == 2)))

        # ---- merge accumulators ----
        nc.vector.tensor_add(outT_A[:], outT_A[:], poG["G3"][:])
        nc.vector.tensor_add(outT_B[:, 0:NI], outT_B[:, 0:NI], poG["G4"][0][:])
        nc.vector.tensor_add(outT_B[:, NI:IP], outT_B[:, NI:IP], poG["G4"][1][:])

        # ---- finals ----
        def finals(outT_sb, ipass):
            ob = fin_pool.tile([128, IP // 128, DF], F32, tag="ob")
            for c in range(IP // 128):
                pf = ppsum.tile([128, DF + 1], F32, tag="ps")
                nc.tensor.transpose(
                    pf[:], outT_sb[:, c * 128:(c + 1) * 128],
                    identf[0:DF + 1, 0:DF + 1])
                rcp = fin_pool.tile([128, 1], F32, tag="rcp")
                nc.vector.reciprocal(rcp[:], pf[:, DF:DF + 1])
                nc.vector.tensor_scalar_mul(ob[:, c, :], pf[:, 0:DF], rcp[:])
            nc.sync.dma_start(
                out[ipass * IP:(ipass + 1) * IP, :].rearrange(
                    "(c p) f -> p c f", p=128),
                ob[:])

        finals(outT_A, 0)
        finals(outT_B, 1)


_COMPILED = None


def get_compiled():
    global _COMPILED
    if _COMPILED is None:
        nc = bacc.Bacc("TRN2", target_bir_lowering=False, debug=False,
                       enable_asserts=False, num_devices=NCORES)
        with tile.TileContext(nc) as tc:
            build_kernel(tc)
        nc.compile()
        _COMPILED = nc
    return _COMPILED


def _to_pair_major(xT):
    """[DM, s_len] fp32 -> fp8 [128, nblk * 4 pairs * 2 slots * 512]."""
    s_len = xT.shape[1]
    nblk = s_len // NI
    # (pair, slot, p, blk, col) -> (p, blk, pair, slot, col)
    r = xT.reshape(MCP, 2, 128, nblk, NI).transpose(2, 3, 0, 1, 4)
    return np.ascontiguousarray(r.reshape(128, nblk * MCP * 2 * NI)).astype(NP_F8)


def _to_block_major(xT):
    """[DM, s_len] -> bf16 [128, nblk*MC*NI]: 512-col blocks, m-chunk-major."""
    s_len = xT.shape[1]
    nblk = s_len // NI
    return np.ascontiguousarray(
        xT.reshape(MC, 128, nblk, NI).transpose(1, 2, 0, 3)
        .reshape(128, nblk * MC * NI)).astype(NP_BF16)


def _w_pair_major(W):
    """W [64, DM] fp32 -> fp8 [128, 4 pairs * 2 slots * 128] with [w|w] dup."""
    WT = np.ascontiguousarray(W.T)                    # [DM, 64]
    dup = np.concatenate([WT, WT], axis=1)            # [DM, 128]
    r = dup.reshape(MCP, 2, 128, 128).transpose(2, 0, 1, 3)
    return np.ascontiguousarray(r.reshape(128, MCP * 2 * 128)).astype(NP_F8)


def make_in_maps(queries, keys, values, Wq, bq, Wk, bk, Wv, bv):
    queries = np.asarray(queries, dtype=np.float32)
    keys = np.asarray(keys, dtype=np.float32)
    values = np.asarray(values, dtype=np.float32)
    wq_host = _w_pair_major(np.asarray(Wq, np.float32))
    wk_host = _w_pair_major(np.asarray(Wk, np.float32))
    WvT = np.asarray(Wv, np.float32).T                # [DM, 64]
    wv_host = np.ascontiguousarray(
        WvT.reshape(MC, 128, DF).transpose(1, 0, 2).reshape(128, MC * DF)
    ).astype(NP_BF16)
    bias64 = np.stack(
        [np.asarray(bq), np.asarray(bk), np.asarray(bv)], axis=1
    ).astype(np.float32)
    bias_host = np.concatenate([bias64, bias64], axis=0)  # [128, 3]

    in_maps = []
    for c in range(NCORES):
        b, h = c // 2, c % 2
        in_maps.append({
            "xq": _to_pair_major(queries[b, h * SQ:(h + 1) * SQ, :].T),
            "xk": _to_pair_major(keys[b].T),
            "xv": _to_block_major(values[b].T),
            "wq": wq_host, "wk": wk_host, "wv": wv_host, "bias": bias_host,
        })
    return in_maps


def assemble(results):
    out = np.zeros((B, S, DF), dtype=np.float32)
    for c in range(NCORES):
        b, h = c // 2, c % 2
        out[b, h * SQ:(h + 1) * SQ, :] = results[c]["out"]
    return out


def kernel(**inputs):
    nc = get_compiled()
    in_maps = make_in_maps(**inputs)
    res = run_bass_kernel_spmd(nc, in_maps, core_ids=list(range(NCORES)))
    return assemble(res.results)


# revision 7
# speedup vs baseline: 1.1583x; 1.0716x over previous
"""Distributed Trainium2 Bass kernel for a single attention head.

Reference computation (fp32 jax):
    q = queries @ Wq.T + bq        # [B,S,Df]
    k = keys    @ Wk.T + bk
    v = values  @ Wv.T + bv
    attn = softmax((q @ k.T) / sqrt(Df), axis=-1)
    out  = attn @ v                # [B,S,Df]

with B=4, S=4096, D_MODEL=1024, D_FEATURE=64.

Sharding: 8 cores = (batch b in 0..3) x (query-half h in 0..1).
Core c handles batch b=c//2, q rows [h*2048, (h+1)*2048). Each core gets
its q-half plus the FULL keys/values of its batch (no collectives).

v3 design notes. The ACT engine is the kernel's floor: 64 exp
instructions of [128,1024] (~70us busy, ~1.15us each with semaphore
latency). Everything else is arranged to keep that stream gap-free:
  - q/k projections use fp8e4 inputs (half the DMA bytes; PE runs fp8
    at bf16 element rate, so the 4-matmul DoubleRow form is kept only
    because it needs half the instructions). v stays bf16: fp8 noise
    on the v path lands on the output un-averaged and busts the 2e-2
    budget.
  - DMA order puts wq/wk, q, and k-block-0 first; kb0 emits only the
    i-half-A scores/exp so the first exps issue as soon as q-blocks
    0-1 land (the B half of chunks 0..3 is re-emitted during kb1-2).
  - inside a pair the B-half scores are queued BEFORE the A-half
    attn@v matmuls: the PE FIFO is strictly in-order, and the scores
    only wait on the exp that frees their psum buffer, so ACT sees
    back-to-back work.
  - attn@v runs in 4 accumulation generations (G1=A/keys0-15,
    G2=B/keys0-15, G3=A/keys16-31 in the single opsum slot;
    G4=B/keys16-31 in the two ppsum banks the projections free after
    kb7). Replayed generations consume SBUF-parked exp outputs and
    are drip-fed through a filler queue, at most 6 matmuls after each
    exp group, so they never displace scores from the PE FIFO.
  - output is written p-major ([128, 1024] f32, 2KB rows) so the
    final DMA is 128 fat descriptors instead of 2048 256-byte ones;
    the host undoes the permutation.
"""

import numpy as np
import ml_dtypes

import concourse.bass as bass
import concourse.mybir as mybir
import concourse.tile as tile
from concourse import bacc
from concourse.bass_utils import run_bass_kernel_spmd
from concourse.masks import make_identity

B = 4
S = 4096
DM = 1024
DF = 64
NCORES = 8
SQ = S // 2          # local q rows per core
MC = DM // 128       # 8 contraction chunks
MCP = MC // 2        # 4 DoubleRow chunk-pairs
NI = 512             # moving-operand tile (one PSUM bank of fp32)
JC = S // 128        # 32 key chunks
NBQ = SQ // NI       # 4 q column blocks
NBK = S // NI        # 8 k/v column blocks
IP = SQ // 2         # 1024: i-rows per attention half
QW = MCP * 2 * NI    # 4096 fp8 elements per q/k block
BF16 = mybir.dt.bfloat16
F8 = mybir.dt.float8e4
F32 = mybir.dt.float32
NP_BF16 = ml_dtypes.bfloat16
NP_F8 = ml_dtypes.float8_e4m3
EXP = mybir.ActivationFunctionType.Exp
DR = mybir.MatmulPerfMode.DoubleRow

# parked exp outputs: all 32 B-half chunks + A-half chunks 16..23
NPARK = JC + 8


def _park_idx(jc, ipass):
    if ipass == 1:
        return jc
    assert 16 <= jc < 24
    return JC + (jc - 16)


def build_kernel(tc):
    nc = tc.nc
    xq = nc.dram_tensor("xq", [128, NBQ * QW], F8, kind="ExternalInput")
    xk = nc.dram_tensor("xk", [128, NBK * QW], F8, kind="ExternalInput")
    xv = nc.dram_tensor("xv", [128, NBK * MC * NI], BF16, kind="ExternalInput")
    wqk = nc.dram_tensor("wqk", [128, 2 * MCP * 2 * 128], F8, kind="ExternalInput")
    wv = nc.dram_tensor("wv", [128, MC * DF], BF16, kind="ExternalInput")
    bias = nc.dram_tensor("bias", [128, 3], F32, kind="ExternalInput")
    out = nc.dram_tensor("out", [128, 2 * (IP // 128) * DF], F32,
                         kind="ExternalOutput")

    from contextlib import ExitStack

    with ExitStack() as ctx:
        const_pool = ctx.enter_context(tc.tile_pool(name="const", bufs=1))
        xq_pool = ctx.enter_context(tc.tile_pool(name="xq", bufs=2))
        xk0_pool = ctx.enter_context(tc.tile_pool(name="xk0", bufs=1))
        xk_pool = ctx.enter_context(tc.tile_pool(name="xk", bufs=3))
        xv_pool = ctx.enter_context(tc.tile_pool(name="xv", bufs=3))
        act_pool = ctx.enter_context(tc.tile_pool(name="act", bufs=1))
        pt_pool = ctx.enter_context(tc.tile_pool(name="pt", bufs=4))
        outT_pool = ctx.enter_context(tc.tile_pool(name="outT", bufs=2))
        fin_pool = ctx.enter_context(tc.tile_pool(name="fin", bufs=2))
        # PSUM budget (8 banks): ppsum 2x[<=128,512] = 2 banks (proj
        # psum + vtrans scratch, then the G4 accumulator pair + finals
        # scratch), spsum 2x[128,1024] = 4 banks (scores double-buffer),
        # opsum [65,1024] = 2 banks (G1/G2/G3 in sequence).
        ppsum = ctx.enter_context(tc.tile_pool(name="ppsum", bufs=2, space="PSUM"))
        spsum = ctx.enter_context(tc.tile_pool(name="spsum", bufs=2, space="PSUM"))
        opsum = ctx.enter_context(tc.tile_pool(name="opsum", bufs=1, space="PSUM"))

        # ---- DMAs, most-critical first: the first-exp path is
        # wqk + bias + xq01 + xk0; wv + xv0 right behind for the kb0
        # v projection ----
        wqk_sb = const_pool.tile([128, 2 * MCP * 2 * 128], F8, tag="wqk")
        nc.sync.dma_start(wqk_sb[:], wqk[:])
        bias_sb = const_pool.tile([128, 3], F32, tag="bias")
        nc.sync.dma_start(bias_sb[:], bias[:])
        q_t01 = xq_pool.tile([128, 2 * QW], F8, tag="xq")
        nc.sync.dma_start(q_t01[:], xq[:, 0:2 * QW])
        k_t0 = xk0_pool.tile([128, QW], F8, tag="xk0")
        nc.sync.dma_start(k_t0[:], xk[:, 0:QW])
        wv_sb = const_pool.tile([128, MC * DF], BF16, tag="wv")
        nc.sync.dma_start(wv_sb[:], wv[:])

        k_tiles = {0: k_t0}
        v_tiles = {}

        def load_k(i):
            t = xk_pool.tile([128, QW], F8, tag="xk")
            nc.sync.dma_start(t[:], xk[:, i * QW:(i + 1) * QW])
            k_tiles[i] = t

        def load_v(i):
            t = xv_pool.tile([128, MC * NI], BF16, tag="xv")
            nc.sync.dma_start(t[:], xv[:, i * MC * NI:(i + 1) * MC * NI])
            v_tiles[i] = t

        load_v(0)
        q_t23 = xq_pool.tile([128, 2 * QW], F8, tag="xq")
        nc.sync.dma_start(q_t23[:], xq[:, 2 * QW:4 * QW])
        for i in range(1, NBK):
            load_k(i)
            load_v(i)

        def q_ap(i):
            t = q_t01 if i < 2 else q_t23
            return t[:, (i % 2) * QW:(i % 2 + 1) * QW]

        # ---- constants computed on-chip ----
        scratch = const_pool.tile([DF, 1], F32, tag="scratch")
        ident = const_pool.tile([128, 128], BF16, tag="ident")
        make_identity(nc, ident[:])
        identf = const_pool.tile([128, 128], F32, tag="identf")
        make_identity(nc, identf[:])
        # preload the ACT exp table while DMAs stream
        nc.scalar.activation(scratch[:], identf[0:DF, 0:1], EXP)

        # ---- PE warm-up: dummy matmuls cover the HAM ramp until the
        # first q-projection's input DMA lands ----
        warm = opsum.tile([DF, 128], F32, tag="po")
        for _ in range(44):
            nc.tensor.matmul(warm[:], ident[:, 0:DF], ident[:], start=True, stop=True)

        # ---- persistent activations ----
        qT_sb = act_pool.tile([128, SQ], BF16, tag="qT")
        kT_sb = act_pool.tile([128, S], BF16, tag="kT")
        vT_sb = act_pool.tile([DF, S], BF16, tag="vT")
        v_sb = act_pool.tile([128, JC * (DF + 1)], BF16, tag="v")  # [128, 32*65]
        nc.gpsimd.memset(v_sb[:], 1.0)  # col DF of every block stays 1.0
        park_sb = act_pool.tile([128, NPARK * IP], BF16, tag="park")  # 10 MB

        def project_block_qk(x_ap, i, w_off, dest_sb, bias_col):
            """One 512-col q/k projection block: 4 fp8 DoubleRow matmuls."""
            ps = ppsum.tile([128, NI], F32, tag="ps")
            for p in range(MCP):
                o = w_off + p * 256
                nc.tensor.matmul(
                    ps[:],
                    wqk_sb[:, o:o + 256].rearrange("p (s c) -> p s c", s=2),
                    x_ap[:, p * 2 * NI:(p + 1) * 2 * NI].rearrange(
                        "p (s c) -> p s c", s=2),
                    start=(p == 0), stop=(p == MCP - 1),
                    perf_mode=DR,
                )
            nc.vector.tensor_scalar_add(
                dest_sb[:, i * NI:(i + 1) * NI], ps[:],
                bias_sb[0:128, bias_col:bias_col + 1])

        def project_block_v(x_tile, i):
            """One 512-col v projection block: 8 bf16 matmuls."""
            ps = ppsum.tile([DF, NI], F32, tag="ps")
            for mc_i in range(MC):
                nc.tensor.matmul(
                    ps[:], wv_sb[:, mc_i * DF:(mc_i + 1) * DF],
                    x_tile[:, mc_i * NI:(mc_i + 1) * NI],
                    start=(mc_i == 0), stop=(mc_i == MC - 1),
                )
            nc.vector.tensor_scalar_add(
                vT_sb[:, i * NI:(i + 1) * NI], ps[:], bias_sb[0:DF, 2:3])

        # ---- q blocks 0,1 immediately (blocks 2,3 are only needed by
        # the deferred B halves, so they are emitted at the end of kb0
        # once their DMA has had time) ----
        project_block_qk(q_ap(0), 0, 0, qT_sb, 0)
        project_block_qk(q_ap(1), 1, 0, qT_sb, 0)

        def v_slice(jc):
            return v_sb[:, jc * (DF + 1):jc * (DF + 1) + DF + 1]

        # ---- filler queue: parked attn@v matmuls drip-fed between
        # exp groups so they never displace scores in the PE FIFO ----
        filler_q = []

        def pop_fillers(n=6):
            for _ in range(min(n, len(filler_q))):
                filler_q.pop(0)()

        poG = {}
        outT_A = outT_pool.tile([DF + 1, IP], F32, tag="outT", name="outT_A")
        outT_B = outT_pool.tile([DF + 1, IP], F32, tag="outT", name="outT_B")

        def attnv_mm(po, jc, src_ap, ii, first, last, po_col=None):
            nc.tensor.matmul(
                po[:, ii * NI:(ii + 1) * NI] if po_col is None else po[:],
                v_slice(jc),
                src_ap[:, ii * NI:(ii + 1) * NI],
                start=first, stop=last,
            )

        def push_gen(gen, jcs, jc_first, jc_last):
            for jc in jcs:
                src = park_sb[:, _park_idx(jc, 1) * IP:(_park_idx(jc, 1) + 1) * IP]
                for ii in range(2):
                    if gen == "G4":
                        filler_q.append(
                            (lambda jc=jc, ii=ii, src=src: nc.tensor.matmul(
                                poG["G4"][ii][:], v_slice(jc),
                                src[:, ii * NI:(ii + 1) * NI],
                                start=(jc == jc_first), stop=(jc == jc_last))))
                    else:
                        filler_q.append(
                            (lambda gen=gen, jc=jc, ii=ii, src=src: attnv_mm(
                                poG[gen], jc, src, ii,
                                (jc == jc_first), (jc == jc_last))))

        def hooks(jc, ipass, pts):
            if ipass == 1:
                if jc >= 28:
                    for ii in range(2):
                        nc.tensor.matmul(
                            poG["G4"][ii][:], v_slice(jc),
                            pts[:, ii * NI:(ii + 1) * NI],
                            start=False, stop=(jc == 31))
                return
            if jc < 16:
                for ii in range(2):
                    attnv_mm(poG["G1"], jc, pts, ii, (jc == 0), (jc == 15))
            elif jc >= 24:
                for ii in range(2):
                    attnv_mm(poG["G3"], jc, pts, ii, False, (jc == 31))

        def attn_pair(jc0, ipasses=(0, 1), mid=None):
            """Scores + exp for the requested i-halves of chunks jc0, jc0+1.
            Emission order per half: scores, exps, fillers; the NEXT
            half's scores go into the PE FIFO before this half's attn@v
            hooks so ACT never waits on FIFO-ordering. `mid` is emitted
            once, right after the first half's exps."""
            pending = None  # hook args from the previous half
            for ipass in ipasses:
                io = ipass * IP
                ss0 = spsum.tile([128, IP], F32, tag="ss", name="ss0")
                ss1 = spsum.tile([128, IP], F32, tag="ss", name="ss1")
                sss = [ss0, ss1]
                for ii in range(IP // NI):
                    for t in range(2):
                        jc = jc0 + t
                        p0 = t * DF
                        nc.tensor.matmul(
                            sss[t][:, ii * NI:(ii + 1) * NI],
                            kT_sb[p0:p0 + DF, jc * 128:(jc + 1) * 128],
                            qT_sb[p0:p0 + DF, io + ii * NI:io + (ii + 1) * NI],
                            start=True, stop=True,
                        )
                if pending:
                    for args in pending:
                        hooks(*args)
                    pending = None
                group = []
                for t in range(2):
                    jc = jc0 + t
                    parked = (ipass == 1) or (16 <= jc < 24)
                    if parked:
                        idx = _park_idx(jc, ipass)
                        pts = park_sb[:, idx * IP:(idx + 1) * IP]
                    else:
                        pts = pt_pool.tile([128, IP], BF16, tag="pt")
                    nc.scalar.activation(pts[:], sss[t][:], EXP, scale=0.125)
                    group.append((jc, ipass, pts))
                if mid is not None:
                    mid()
                    mid = None
                pop_fillers()
                pending = group
            for args in pending:
                hooks(*args)

        def vtrans(jc):
            pv = ppsum.tile([128, DF], BF16, tag="ps")
            nc.tensor.transpose(
                pv[:], vT_sb[:, jc * 128:(jc + 1) * 128], ident[0:DF, 0:DF])
            nc.vector.tensor_copy(
                v_sb[:, jc * (DF + 1):jc * (DF + 1) + DF], pv[:])

        # ================= kb0: get ACT started ASAP =================
        # FIFO order: kproj0, scores+exp(0,1 A-half), then the v
        # pipeline (which stalls on the later xv0 DMA) before the
        # attn@v hooks that need it.
        poG["G1"] = opsum.tile([DF + 1, IP], F32, tag="po", name="poG1")
        project_block_qk(k_tiles[0], 0, MCP * 2 * 128, kT_sb, 1)

        def kb0_mid():
            project_block_v(v_tiles[0], 0)
            for jc in range(4):
                vtrans(jc)

        attn_pair(0, ipasses=(0,), mid=kb0_mid)
        attn_pair(2, ipasses=(0,))
        project_block_qk(q_ap(2), 2, 0, qT_sb, 0)
        project_block_qk(q_ap(3), 3, 0, qT_sb, 0)
        deferred_b = [0, 2]

        # ================= kb loop 1..7 =================
        for kb in range(1, NBK):
            project_block_qk(k_tiles[kb], kb, MCP * 2 * 128, kT_sb, 1)
            project_block_v(v_tiles[kb], kb)
            for jc in range(4 * kb, 4 * kb + 4):
                vtrans(jc)

            if kb == 4:
                # G1 complete: evict and hand the opsum slot to G2
                nc.vector.tensor_copy(outT_A[:], poG["G1"][:])
                poG["G2"] = opsum.tile([DF + 1, IP], F32, tag="po", name="poG2")
                push_gen("G2", range(0, 16), 0, 15)
            if kb == 6:
                # G2 complete: evict, slot to G3; queue parked A 16..23
                nc.vector.tensor_copy(outT_B[:], poG["G2"][:])
                poG["G3"] = opsum.tile([DF + 1, IP], F32, tag="po", name="poG3")
                for jc in range(16, 24):
                    idx = _park_idx(jc, 0)
                    src = park_sb[:, idx * IP:(idx + 1) * IP]
                    for ii in range(2):
                        filler_q.append(
                            (lambda jc=jc, ii=ii, src=src: attnv_mm(
                                poG["G3"], jc, src, ii, (jc == 16), False)))
            if kb == 7:
                # projections are done: the ppsum banks become the two
                # G4 accumulator halves
                g4a = ppsum.tile([DF + 1, NI], F32, tag="ps", name="g4a")
                g4b = ppsum.tile([DF + 1, NI], F32, tag="ps", name="g4b")
                poG["G4"] = [g4a, g4b]
                push_gen("G4", range(16, 28), 16, 31)

            for jc0 in range(4 * kb, 4 * kb + 4, 2):
                attn_pair(jc0)
                if deferred_b:
                    attn_pair(deferred_b.pop(0), ipasses=(1,))
        assert not filler_q and not deferred_b

        # ---- merge accumulators ----
        nc.vector.tensor_add(outT_A[:], outT_A[:], poG["G3"][:])
        nc.vector.tensor_add(outT_B[:, 0:NI], outT_B[:, 0:NI], poG["G4"][0][:])
        nc.vector.tensor_add(outT_B[:, NI:IP], outT_B[:, NI:IP], poG["G4"][1][:])

        # ---- finals ----
        def finals(outT_sb, ipass):
            ob = fin_pool.tile([128, (IP // 128) * DF], F32, tag="ob")
            for c in range(IP // 128):
                pf = ppsum.tile([128, DF + 1], F32, tag="ps")
                nc.tensor.transpose(
                    pf[:], outT_sb[:, c * 128:(c + 1) * 128],
                    identf[0:DF + 1, 0:DF + 1])
                rcp = fin_pool.tile([128, 1], F32, tag="rcp")
                nc.vector.reciprocal(rcp[:], pf[:, DF:DF + 1])
                nc.vector.tensor_scalar_mul(
                    ob[:, c * DF:(c + 1) * DF], pf[:, 0:DF], rcp[:])
            half = (IP // 128) * DF
            nc.sync.dma_start(out[:, ipass * half:(ipass + 1) * half], ob[:])

        finals(outT_A, 0)
        finals(outT_B, 1)


_COMPILED = None


def get_compiled():
    global _COMPILED
    if _COMPILED is None:
        nc = bacc.Bacc("TRN2", target_bir_lowering=False, debug=False,
                       enable_asserts=False, num_devices=NCORES)
        with tile.TileContext(nc) as tc:
            build_kernel(tc)
        nc.compile()
        _COMPILED = nc
    return _COMPILED


def _to_pair_major(xT):
    """[DM, s_len] fp32 -> fp8 [128, nblk * 4 pairs * 2 slots * 512]."""
    s_len = xT.shape[1]
    nblk = s_len // NI
    r = xT.reshape(MCP, 2, 128, nblk, NI).transpose(2, 3, 0, 1, 4)
    return np.ascontiguousarray(r.reshape(128, nblk * MCP * 2 * NI)).astype(NP_F8)


def _to_block_major(xT):
    """[DM, s_len] -> bf16 [128, nblk*MC*NI]: 512-col blocks, m-chunk-major."""
    s_len = xT.shape[1]
    nblk = s_len // NI
    return np.ascontiguousarray(
        xT.reshape(MC, 128, nblk, NI).transpose(1, 2, 0, 3)
        .reshape(128, nblk * MC * NI)).astype(NP_BF16)


def _w_pair_major(W):
    """W [64, DM] fp32 -> fp8 [128, 4 pairs * 2 slots * 128] with [w|w] dup."""
    WT = np.ascontiguousarray(np.asarray(W, np.float32).T)   # [DM, 64]
    dup = np.concatenate([WT, WT], axis=1)                   # [DM, 128]
    r = dup.reshape(MCP, 2, 128, 128).transpose(2, 0, 1, 3)
    return np.ascontiguousarray(r.reshape(128, MCP * 2 * 128)).astype(NP_F8)


def make_in_maps(queries, keys, values, Wq, bq, Wk, bk, Wv, bv):
    queries = np.asarray(queries, dtype=np.float32)
    keys = np.asarray(keys, dtype=np.float32)
    values = np.asarray(values, dtype=np.float32)
    wqk_host = np.concatenate([_w_pair_major(Wq), _w_pair_major(Wk)], axis=1)
    WvT = np.asarray(Wv, np.float32).T                       # [DM, 64]
    wv_host = np.ascontiguousarray(
        WvT.reshape(MC, 128, DF).transpose(1, 0, 2).reshape(128, MC * DF)
    ).astype(NP_BF16)
    bias64 = np.stack(
        [np.asarray(bq), np.asarray(bk), np.asarray(bv)], axis=1
    ).astype(np.float32)
    bias_host = np.concatenate([bias64, bias64], axis=0)     # [128, 3]

    in_maps = []
    for c in range(NCORES):
        b, h = c // 2, c % 2
        in_maps.append({
            "xq": _to_pair_major(queries[b, h * SQ:(h + 1) * SQ, :].T),
            "xk": _to_pair_major(keys[b].T),
            "xv": _to_block_major(values[b].T),
            "wqk": wqk_host, "wv": wv_host, "bias": bias_host,
        })
    return in_maps


def assemble(results):
    out = np.zeros((B, S, DF), dtype=np.float32)
    for c in range(NCORES):
        b, h = c // 2, c % 2
        # [128, 2*8*64] p-major -> [2048, 64]
        arr = results[c]["out"].reshape(128, 2, IP // 128, DF)
        out[b, h * SQ:(h + 1) * SQ, :] = (
            arr.transpose(1, 2, 0, 3).reshape(SQ, DF))
    return out


def kernel(**inputs):
    nc = get_compiled()
    in_maps = make_in_maps(**inputs)
    res = run_bass_kernel_spmd(nc, in_maps, core_ids=list(range(NCORES)))
    return assemble(res.results)
